# revision 5
# baseline (speedup 1.0000x reference)
"""Embedding-bag (sum over sentence dim) kernel for Trainium2, 8 NeuronCores.

Problem: two embedding tables [100000, 64] f32, two index tensors [4096, 200]
int32/int64; output = (sum_s emb_pri[idx_pri[b,s]], sum_s emb_sec[idx_sec[b,s]])
as two [4096, 64] f32 tensors.

Strategy (all measured on HW):
- Data-parallel over batch: each core handles 512 rows for both tables.
- The only fast random-gather primitive is the custom-ucode `dma_gather`
  (InstDMAGatherAnt): ~2.3 ns/row with 4 parallel SWDGE queues, but indices
  are int16 and num_idxs <= 1024 per instruction. The 100k vocab is split
  into 4 chunks of 25000 rows (+1 zero pad row each); tokens are bucketed
  per chunk on the host and padded per 128-row block to the block's max
  per-row chunk count (pads point at the zero row, contributing 0 to sums).
- dma_gather places stream position i at SBUF [i%128, i//128, :], so a
  stream packed column-major (lane p = batch row p of the block) lands each
  batch row's tokens in one partition; an in-place DVE binary-tree add then
  reduces the K token-columns to [128, 64], accumulated over the 4 chunks.
- Raw bacc program with an explicit semaphore pipeline:
    sync:   gidx load, output DMAs
    gpsimd: 4-queue rotated dma_gathers into a ring of dest tiles
    vector: tree-reduce + chunk accumulation, slot recycling
"""

import sys

if "/opt/trn_rl_repo" not in sys.path:
    sys.path.insert(0, "/opt/trn_rl_repo")

from contextlib import ExitStack

import numpy as np

import concourse.bacc as bacc
import concourse.bass as bass
import concourse.mybir as mybir
from concourse import library_config
from concourse.bass_utils import run_bass_kernel_spmd

N_CORES = 8
P = 128
VOCAB = 100000
SEQ = 200
DIM = 64
BATCH = 4096

N_CHUNKS = 4
CHUNK = VOCAB // N_CHUNKS          # 25000
CHUNK_ROWS = CHUNK + 1             # + zero pad row
PAD_IDX = CHUNK                    # local index of the zero row
GMAX = 1024                        # max num_idxs per dma_gather (HW ring limit)
GCOLS = GMAX // P                  # 8 dest columns per gather
NBUF = 4                           # dest tile ring depth
NQ = 4                             # SWDGE queues


def build_nc(K, n_blocks):
    """Build the SPMD per-core program.

    K: int array [2, N_CHUNKS, n_blocks] -- token columns per job, each a
       multiple of GCOLS; identical across cores (max over cores).
    """
    kmax = int(K.max())
    total_cols = int(K.sum())
    idx_cols = total_cols * P // 16  # int16 columns of the wrapped index tensor

    nc = bacc.Bacc("TRN2", debug=False, num_swdge_queues=NQ)

    emb_cat = nc.dram_tensor(
        "emb_cat", [2 * N_CHUNKS * CHUNK_ROWS, DIM], mybir.dt.float32,
        kind="ExternalInput",
    )
    gidx = nc.dram_tensor("gidx", [P, idx_cols], mybir.dt.int16, kind="ExternalInput")
    out_pri = nc.dram_tensor("out_pri", [n_blocks * P, DIM], mybir.dt.float32, kind="ExternalOutput")
    out_sec = nc.dram_tensor("out_sec", [n_blocks * P, DIM], mybir.dt.float32, kind="ExternalOutput")
    outs = (out_pri, out_sec)

    # job order: (T, b) outer, chunk k inner so chunk partials accumulate
    jobs = [(t, b, k) for t in range(2) for b in range(n_blocks) for k in range(N_CHUNKS)]

    with (
        nc.Block() as _block,
        nc.sbuf_tensor("gidx_sb", [P, idx_cols], mybir.dt.int16) as gidx_sb,
        nc.semaphore("io") as io,
        ExitStack() as stack,
    ):
        slots = [
            stack.enter_context(
                nc.sbuf_tensor(f"slot{i}", [P, kmax * DIM], mybir.dt.float32)
            )
            for i in range(NBUF)
        ]
        accs = [
            stack.enter_context(
                nc.sbuf_tensor(f"acc{t}_{b}", [P, DIM], mybir.dt.float32)
            )
            for t in range(2)
            for b in range(n_blocks)
        ]
        done = [stack.enter_context(nc.semaphore(f"done{i}")) for i in range(NBUF)]
        free = [stack.enter_context(nc.semaphore(f"free{i}")) for i in range(NBUF)]
        oready = stack.enter_context(nc.semaphore("oready"))
        vchain = stack.enter_context(nc.semaphore("vchain"))

        # ---- sync engine: load the packed indices
        nc.sync.dma_start(gidx_sb[:], gidx[:]).then_inc(io, 16)

        # ---- gpsimd: all gathers
        nc.gpsimd.load_library(library_config.mlp)
        nc.gpsimd.wait_ge(io, 16)
        gctr = 0          # global gather counter (queue rotation + idx cols)
        done_target = [0] * NBUF
        for j, (t, b, k) in enumerate(jobs):
            slot = j % NBUF
            if j >= NBUF:
                nc.gpsimd.wait_ge(free[slot], j // NBUF)
            kj = int(K[t, k, b])
            src = emb_cat[(t * N_CHUNKS + k) * CHUNK_ROWS:(t * N_CHUNKS + k + 1) * CHUNK_ROWS, :]
            g3 = slots[slot][:].rearrange("p (c d) -> p c d", d=DIM)
            for i in range(kj // GCOLS):
                nc.gpsimd.dma_gather(
                    g3[:, i * GCOLS:(i + 1) * GCOLS, :],
                    src,
                    gidx_sb[:, gctr * (GMAX // 16):(gctr + 1) * (GMAX // 16)],
                    GMAX,
                    GMAX,
                    DIM,
                    queue_num=slot % NQ,
                ).then_inc(done[slot], 16)
                gctr += 1
            done_target[slot] += 16 * (kj // GCOLS)
            # vector consumes this job when done[slot] >= its running target
            jobs[j] = (t, b, k, slot, done_target[slot], kj)

        # ---- vector: tree-reduce each job, accumulate chunks, signal outputs.
        # DVE has an 8-deep exec queue, so same-engine RAW chains need explicit
        # serialization: every op incs vchain and waits for the previous one.
        vc = 0
        for j, (t, b, k, slot, tgt, kj) in enumerate(jobs):
            nc.vector.wait_ge(done[slot], tgt)
            g = slots[slot]
            n = kj
            while n > 1:
                h = n // 2
                if vc:
                    nc.vector.wait_ge(vchain, vc)
                nc.vector.tensor_add(
                    out=g[:, : h * DIM],
                    in0=g[:, : h * DIM],
                    in1=g[:, (n - h) * DIM : n * DIM],
                ).then_inc(vchain, 1)
                vc += 1
                n -= h
            acc = accs[t * n_blocks + b]
            nc.vector.wait_ge(vchain, vc)
            if k == 0:
                nc.vector.tensor_copy(out=acc[:], in_=g[:, :DIM]).then_inc(vchain, 1)
            else:
                nc.vector.tensor_add(out=acc[:], in0=acc[:], in1=g[:, :DIM]).then_inc(vchain, 1)
            vc += 1
            nc.vector.wait_ge(vchain, vc)
            nc.vector.sem_inc(free[slot], 1)
            if k == N_CHUNKS - 1:
                nc.vector.sem_inc(oready, 1)

        # ---- sync engine: write outputs as accs complete
        m = 0
        for t in range(2):
            for b in range(n_blocks):
                m += 1
                nc.sync.wait_ge(oready, m)
                nc.sync.dma_start(
                    out=outs[t][b * P:(b + 1) * P, :],
                    in_=accs[t * n_blocks + b][:],
                ).then_inc(io, 16)
        nc.sync.wait_ge(io, 16 + m * 16)

    nc.compile()
    return nc


def _pack_core(idx_by_table, K, n_blocks):
    """Build one core's wrapped int16 index tensor.

    idx_by_table: [2, bc, SEQ] int32 core-local indices.
    Returns gidx [P, K.sum()*P//16] int16.
    """
    streams = []
    for t in range(2):
        for b in range(n_blocks):
            rows = idx_by_table[t][b * P:(b + 1) * P]  # [P, SEQ]
            for k in range(N_CHUNKS):
                kj = int(K[t, k, b])
                mask = (rows // CHUNK) == k
                local = (rows - k * CHUNK).astype(np.int64)
                order = np.argsort(~mask, axis=1, kind="stable")
                sortloc = np.take_along_axis(local, order, axis=1)
                cnt = mask.sum(axis=1)
                pad_cols = max(kj - SEQ, 0)
                if pad_cols:
                    sortloc = np.concatenate(
                        [sortloc, np.zeros((P, pad_cols), np.int64)], axis=1
                    )
                sel = sortloc[:, :kj]
                sel = np.where(np.arange(kj)[None, :] < cnt[:, None], sel, PAD_IDX)
                # stream position i -> (lane i%128, col i//128): column-major
                streams.append(sel.T.ravel())  # [kj * P]
    s = np.concatenate(streams).astype(np.int16)
    wrapped = s.reshape(-1, 16).T  # [16, n/16]
    return np.tile(wrapped, (8, 1)).copy()  # replicate across 16-partition groups


def kernel(inputs_pri, inputs_sec, emb_pri, emb_sec, _trace=False, _trace_kwargs=None):
    inputs_pri = np.ascontiguousarray(np.asarray(inputs_pri, dtype=np.int32))
    inputs_sec = np.ascontiguousarray(np.asarray(inputs_sec, dtype=np.int32))
    emb_pri = np.ascontiguousarray(np.asarray(emb_pri, dtype=np.float32))
    emb_sec = np.ascontiguousarray(np.asarray(emb_sec, dtype=np.float32))

    batch = inputs_pri.shape[0]
    bc = batch // N_CORES
    n_blocks = bc // P

    # emb_cat: [2, 4, 25001, 64] with a zero row per chunk
    emb_cat = np.zeros((2, N_CHUNKS, CHUNK_ROWS, DIM), np.float32)
    for t, emb in enumerate((emb_pri, emb_sec)):
        for k in range(N_CHUNKS):
            emb_cat[t, k, :CHUNK] = emb[k * CHUNK:(k + 1) * CHUNK]
    emb_cat = np.ascontiguousarray(emb_cat.reshape(2 * N_CHUNKS * CHUNK_ROWS, DIM))

    # per-core per-block per-chunk max token counts -> uniform K (multiple of GCOLS)
    per_core = [
        (inputs_pri[c * bc:(c + 1) * bc], inputs_sec[c * bc:(c + 1) * bc])
        for c in range(N_CORES)
    ]
    K = np.zeros((2, N_CHUNKS, n_blocks), np.int64)
    for c in range(N_CORES):
        for t in range(2):
            rows_t = per_core[c][t]
            chunk_of = rows_t // CHUNK  # [bc, SEQ]
            for b in range(n_blocks):
                blk = chunk_of[b * P:(b + 1) * P]
                for k in range(N_CHUNKS):
                    K[t, k, b] = max(K[t, k, b], (blk == k).sum(axis=1).max())
    K = ((K + GCOLS - 1) // GCOLS) * GCOLS
    K = np.maximum(K, GCOLS)

    nc = build_nc(K, n_blocks)

    in_maps = []
    for c in range(N_CORES):
        gidx = _pack_core(per_core[c], K, n_blocks)
        in_maps.append({"emb_cat": emb_cat, "gidx": gidx})

    kwargs = {}
    if _trace:
        kwargs["trace"] = True
        if _trace_kwargs:
            kwargs.update(_trace_kwargs)
    res = run_bass_kernel_spmd(nc, in_maps, list(range(N_CORES)), **kwargs)
    outs = res.results
    out_pri = np.concatenate([outs[c]["out_pri"] for c in range(N_CORES)], axis=0)
    out_sec = np.concatenate([outs[c]["out_sec"] for c in range(N_CORES)], axis=0)
    if _trace:
        return (out_pri, out_sec), res
    return out_pri, out_sec


# revision 8
# speedup vs baseline: 2.1983x; 2.1983x over previous
"""Embedding-bag (sum over sentence dim) kernel for Trainium2, 8 NeuronCores.

Problem: two embedding tables [100000, 64] f32, two index tensors [4096, 200]
int32/int64; output = (sum_s emb_pri[idx_pri[b,s]], sum_s emb_sec[idx_sec[b,s]])
as two [4096, 64] f32 tensors.

Strategy (all measured on HW):
- Data-parallel over batch: each core handles 512 rows for both tables.
- The only fast random-gather primitive is the custom-ucode `dma_gather`
  (InstDMAGatherAnt): ~2.3 ns/row with 4 parallel SWDGE queues, but indices
  are int16 and num_idxs <= 1024 per instruction. The 100k vocab is split
  into 4 chunks of 25000 rows (+1 zero pad row each); tokens are bucketed
  per chunk on the host and padded per 128-row block to the block's max
  per-row chunk count (pads point at the zero row, contributing 0 to sums).
- dma_gather places stream position i at SBUF [i%128, i//128, :], so a
  stream packed column-major (lane p = batch row p of the block) lands each
  batch row's tokens in one partition; an in-place DVE binary-tree add then
  reduces the K token-columns to [128, 64], accumulated over the 4 chunks.
- Raw bacc program with an explicit semaphore pipeline:
    sync:   gidx load, output DMAs
    gpsimd: 4-queue rotated dma_gathers into a ring of dest tiles
    vector: tree-reduce + chunk accumulation, slot recycling
"""

import sys

if "/opt/trn_rl_repo" not in sys.path:
    sys.path.insert(0, "/opt/trn_rl_repo")

from contextlib import ExitStack

import numpy as np

import concourse.bacc as bacc
import concourse.bass as bass
import concourse.mybir as mybir
from concourse import library_config
from concourse.bass_utils import run_bass_kernel_spmd

N_CORES = 8
P = 128
VOCAB = 100000
SEQ = 200
DIM = 64
BATCH = 4096

N_CHUNKS = 4
CHUNK = VOCAB // N_CHUNKS          # 25000
CHUNK_ROWS = CHUNK + 1             # + zero pad row
PAD_IDX = CHUNK                    # local index of the zero row
GMAX = 1024                        # max num_idxs per dma_gather (HW ring limit)
GCOLS = GMAX // P                  # 8 dest columns per gather
NBUF = 4                           # dest tile ring depth
NQ = 4                             # SWDGE queues


def build_nc(K, n_blocks):
    """Build the SPMD per-core program.

    K: int array [2, N_CHUNKS, n_blocks] -- token columns per job, each a
       multiple of GCOLS; identical across cores (max over cores).
    """
    kmax = int(K.max())
    total_cols = int(K.sum())
    idx_cols = total_cols * P // 16  # int16 columns of the wrapped index tensor

    nc = bacc.Bacc("TRN2", debug=False, num_swdge_queues=NQ)

    emb_cat = nc.dram_tensor(
        "emb_cat", [2 * N_CHUNKS * CHUNK_ROWS, DIM], mybir.dt.float32,
        kind="ExternalInput",
    )
    gidx = nc.dram_tensor("gidx", [P, idx_cols], mybir.dt.int16, kind="ExternalInput")
    out_pri = nc.dram_tensor("out_pri", [n_blocks * P, DIM], mybir.dt.float32, kind="ExternalOutput")
    out_sec = nc.dram_tensor("out_sec", [n_blocks * P, DIM], mybir.dt.float32, kind="ExternalOutput")
    outs = (out_pri, out_sec)

    # job order: (T, b) outer, chunk k inner so chunk partials accumulate
    jobs = [(t, b, k) for t in range(2) for b in range(n_blocks) for k in range(N_CHUNKS)]

    with (
        nc.Block() as _block,
        nc.sbuf_tensor("gidx_sb", [P, idx_cols], mybir.dt.int16) as gidx_sb,
        nc.semaphore("io") as io,
        ExitStack() as stack,
    ):
        slots = [
            stack.enter_context(
                nc.sbuf_tensor(f"slot{i}", [P, kmax * DIM], mybir.dt.float32)
            )
            for i in range(NBUF)
        ]
        accs = [
            stack.enter_context(
                nc.sbuf_tensor(f"acc{t}_{b}", [P, DIM], mybir.dt.float32)
            )
            for t in range(2)
            for b in range(n_blocks)
        ]
        # done sems are per (slot, queue): a sem may only be updated from one
        # SWDGE queue, while gathers rotate queues globally for 4-pair overlap.
        done = [
            [stack.enter_context(nc.semaphore(f"done{i}_{q}")) for q in range(NQ)]
            for i in range(NBUF)
        ]
        free = [stack.enter_context(nc.semaphore(f"free{i}")) for i in range(NBUF)]
        oready = stack.enter_context(nc.semaphore("oready"))
        vchain = stack.enter_context(nc.semaphore("vchain"))

        # ---- sync engine: load the packed indices
        nc.sync.dma_start(gidx_sb[:], gidx[:]).then_inc(io, 16)

        # ---- gpsimd: all gathers
        nc.gpsimd.load_library(library_config.mlp)
        nc.gpsimd.wait_ge(io, 16)
        gctr = 0          # global gather counter (queue rotation + idx cols)
        done_target = [[0] * NQ for _ in range(NBUF)]
        for j, (t, b, k) in enumerate(jobs):
            slot = j % NBUF
            if j >= NBUF:
                nc.gpsimd.wait_ge(free[slot], j // NBUF)
            kj = int(K[t, k, b])
            src = emb_cat[(t * N_CHUNKS + k) * CHUNK_ROWS:(t * N_CHUNKS + k + 1) * CHUNK_ROWS, :]
            g3 = slots[slot][:].rearrange("p (c d) -> p c d", d=DIM)
            for i in range(kj // GCOLS):
                q = gctr % NQ
                nc.gpsimd.dma_gather(
                    g3[:, i * GCOLS:(i + 1) * GCOLS, :],
                    src,
                    gidx_sb[:, gctr * (GMAX // 16):(gctr + 1) * (GMAX // 16)],
                    GMAX,
                    GMAX,
                    DIM,
                    queue_num=q,
                ).then_inc(done[slot][q], 16)
                done_target[slot][q] += 16
                gctr += 1
            # vector consumes this job when every queue sem hits its target
            jobs[j] = (t, b, k, slot, tuple(done_target[slot]), kj)

        # ---- vector: tree-reduce each job, accumulate chunks, signal outputs.
        # DVE has an 8-deep exec queue, so same-engine RAW chains need explicit
        # serialization: every op incs vchain and waits for the previous one.
        # DVE executes its own stream in order with per-op DRAIN, so RAW chains
        # on the same engine need no sems (verified on HW); the retire-marker
        # (vchain) gates cross-engine sem_incs on actual retirement.
        vc = 0
        for j, (t, b, k, slot, tgts, kj) in enumerate(jobs):
            for q in range(NQ):
                if tgts[q]:
                    nc.vector.wait_ge(done[slot][q], tgts[q])
            g = slots[slot]
            n = kj
            while n > 1:
                h = n // 2
                nc.vector.tensor_add(
                    out=g[:, : h * DIM],
                    in0=g[:, : h * DIM],
                    in1=g[:, (n - h) * DIM : n * DIM],
                )
                n -= h
            acc = accs[t * n_blocks + b]
            if k == 0:
                nc.vector.tensor_copy(out=acc[:], in_=g[:, :DIM])
            else:
                nc.vector.tensor_add(out=acc[:], in0=acc[:], in1=g[:, :DIM])
            nc.vector.tensor_copy(out=g[:, :4], in_=g[:, :4]).then_inc(vchain, 1)
            vc += 1
            nc.vector.wait_ge(vchain, vc)
            nc.vector.sem_inc(free[slot], 1)
            if k == N_CHUNKS - 1:
                nc.vector.sem_inc(oready, 1)

        # ---- sync engine: write outputs as accs complete
        m = 0
        for t in range(2):
            for b in range(n_blocks):
                m += 1
                nc.sync.wait_ge(oready, m)
                nc.sync.dma_start(
                    out=outs[t][b * P:(b + 1) * P, :],
                    in_=accs[t * n_blocks + b][:],
                ).then_inc(io, 16)
        nc.sync.wait_ge(io, 16 + m * 16)

    nc.compile()
    return nc


def _pack_core(idx_by_table, K, n_blocks):
    """Build one core's wrapped int16 index tensor.

    idx_by_table: [2, bc, SEQ] int32 core-local indices.
    Returns gidx [P, K.sum()*P//16] int16.
    """
    streams = []
    for t in range(2):
        for b in range(n_blocks):
            rows = idx_by_table[t][b * P:(b + 1) * P]  # [P, SEQ]
            for k in range(N_CHUNKS):
                kj = int(K[t, k, b])
                mask = (rows // CHUNK) == k
                local = (rows - k * CHUNK).astype(np.int64)
                order = np.argsort(~mask, axis=1, kind="stable")
                sortloc = np.take_along_axis(local, order, axis=1)
                cnt = mask.sum(axis=1)
                pad_cols = max(kj - SEQ, 0)
                if pad_cols:
                    sortloc = np.concatenate(
                        [sortloc, np.zeros((P, pad_cols), np.int64)], axis=1
                    )
                sel = sortloc[:, :kj]
                sel = np.where(np.arange(kj)[None, :] < cnt[:, None], sel, PAD_IDX)
                # stream position i -> (lane i%128, col i//128): column-major
                streams.append(sel.T.ravel())  # [kj * P]
    s = np.concatenate(streams).astype(np.int16)
    wrapped = s.reshape(-1, 16).T  # [16, n/16]
    return np.tile(wrapped, (8, 1)).copy()  # replicate across 16-partition groups


def kernel(inputs_pri, inputs_sec, emb_pri, emb_sec, _trace=False, _trace_kwargs=None):
    inputs_pri = np.ascontiguousarray(np.asarray(inputs_pri, dtype=np.int32))
    inputs_sec = np.ascontiguousarray(np.asarray(inputs_sec, dtype=np.int32))
    emb_pri = np.ascontiguousarray(np.asarray(emb_pri, dtype=np.float32))
    emb_sec = np.ascontiguousarray(np.asarray(emb_sec, dtype=np.float32))

    batch = inputs_pri.shape[0]
    bc = batch // N_CORES
    n_blocks = bc // P

    # emb_cat: [2, 4, 25001, 64] with a zero row per chunk
    emb_cat = np.zeros((2, N_CHUNKS, CHUNK_ROWS, DIM), np.float32)
    for t, emb in enumerate((emb_pri, emb_sec)):
        for k in range(N_CHUNKS):
            emb_cat[t, k, :CHUNK] = emb[k * CHUNK:(k + 1) * CHUNK]
    emb_cat = np.ascontiguousarray(emb_cat.reshape(2 * N_CHUNKS * CHUNK_ROWS, DIM))

    # per-core per-block per-chunk max token counts -> uniform K (multiple of GCOLS)
    per_core = [
        (inputs_pri[c * bc:(c + 1) * bc], inputs_sec[c * bc:(c + 1) * bc])
        for c in range(N_CORES)
    ]
    K = np.zeros((2, N_CHUNKS, n_blocks), np.int64)
    for c in range(N_CORES):
        for t in range(2):
            rows_t = per_core[c][t]
            chunk_of = rows_t // CHUNK  # [bc, SEQ]
            for b in range(n_blocks):
                blk = chunk_of[b * P:(b + 1) * P]
                for k in range(N_CHUNKS):
                    K[t, k, b] = max(K[t, k, b], (blk == k).sum(axis=1).max())
    K = ((K + GCOLS - 1) // GCOLS) * GCOLS
    K = np.maximum(K, GCOLS)

    nc = build_nc(K, n_blocks)

    in_maps = []
    for c in range(N_CORES):
        gidx = _pack_core(per_core[c], K, n_blocks)
        in_maps.append({"emb_cat": emb_cat, "gidx": gidx})

    kwargs = {}
    if _trace:
        kwargs["trace"] = True
        if _trace_kwargs:
            kwargs.update(_trace_kwargs)
    res = run_bass_kernel_spmd(nc, in_maps, list(range(N_CORES)), **kwargs)
    outs = res.results
    out_pri = np.concatenate([outs[c]["out_pri"] for c in range(N_CORES)], axis=0)
    out_sec = np.concatenate([outs[c]["out_sec"] for c in range(N_CORES)], axis=0)
    if _trace:
        return (out_pri, out_sec), res
    return out_pri, out_sec


# revision 9
# speedup vs baseline: 2.6344x; 1.1984x over previous
"""Embedding-bag (sum over sentence dim) kernel for Trainium2, 8 NeuronCores.

Problem: two embedding tables [100000, 64] f32, two index tensors [4096, 200]
int32/int64; output = (sum_s emb_pri[idx_pri[b,s]], sum_s emb_sec[idx_sec[b,s]])
as two [4096, 64] f32 tensors.

Strategy (all measured on HW):
- Data-parallel over batch: each core handles 512 rows for both tables.
- The only fast random-gather primitive is the custom-ucode `dma_gather`
  (InstDMAGatherAnt): ~2.3 ns/row with 4 parallel SWDGE queues, but indices
  are int16 and num_idxs <= 1024 per instruction. The 100k vocab is split
  into 4 chunks of 25000 rows (+1 zero pad row each); tokens are bucketed
  per chunk on the host and padded per 128-row block to the block's max
  per-row chunk count (pads point at the zero row, contributing 0 to sums).
- dma_gather places stream position i at SBUF [i%128, i//128, :], so a
  stream packed column-major (lane p = batch row p of the block) lands each
  batch row's tokens in one partition; an in-place DVE binary-tree add then
  reduces the K token-columns to [128, 64], accumulated over the 4 chunks.
- Raw bacc program with an explicit semaphore pipeline:
    sync:   gidx load, output DMAs
    gpsimd: 4-queue rotated dma_gathers into a ring of dest tiles
    vector: tree-reduce + chunk accumulation, slot recycling
"""

import sys

if "/opt/trn_rl_repo" not in sys.path:
    sys.path.insert(0, "/opt/trn_rl_repo")

from contextlib import ExitStack

import numpy as np

import concourse.bacc as bacc
import concourse.bass as bass
import concourse.mybir as mybir
from concourse import library_config
from concourse.bass_utils import run_bass_kernel_spmd

N_CORES = 8
P = 128
VOCAB = 100000
SEQ = 200
DIM = 64
BATCH = 4096

N_CHUNKS = 4
CHUNK = VOCAB // N_CHUNKS          # 25000
CHUNK_ROWS = CHUNK + 1             # + zero pad row
PAD_IDX = CHUNK                    # local index of the zero row
GMAX = 1024                        # max num_idxs per dma_gather (HW ring limit)
GCOLS = GMAX // P                  # 8 dest columns per gather
import os as _os
NBUF = int(_os.environ.get("KNBUF", 4))   # dest tile ring depth
NQ = 4                             # SWDGE queues


def build_nc(K, n_blocks):
    """Build the SPMD per-core program.

    K: int array [2, N_CHUNKS, n_blocks] -- token columns per job, each a
       multiple of GCOLS; identical across cores (max over cores).
    """
    kmax = int(K.max())
    total_cols = int(K.sum())
    idx_cols = total_cols * P // 16  # int16 columns of the wrapped index tensor

    nc = bacc.Bacc("TRN2", debug=False, num_swdge_queues=NQ)

    emb_cat = nc.dram_tensor(
        "emb_cat", [2 * N_CHUNKS * CHUNK_ROWS, DIM], mybir.dt.float32,
        kind="ExternalInput",
    )
    gidx = nc.dram_tensor("gidx", [P, idx_cols], mybir.dt.int16, kind="ExternalInput")
    out_pri = nc.dram_tensor("out_pri", [n_blocks * P, DIM], mybir.dt.float32, kind="ExternalOutput")
    out_sec = nc.dram_tensor("out_sec", [n_blocks * P, DIM], mybir.dt.float32, kind="ExternalOutput")
    outs = (out_pri, out_sec)

    # job order: (T, b) outer, chunk k inner so chunk partials accumulate
    jobs = [(t, b, k) for t in range(2) for b in range(n_blocks) for k in range(N_CHUNKS)]

    with (
        nc.Block() as _block,
        nc.sbuf_tensor("gidx_sb", [P, idx_cols], mybir.dt.int16) as gidx_sb,
        nc.semaphore("io") as io,
        ExitStack() as stack,
    ):
        slots = [
            stack.enter_context(
                nc.sbuf_tensor(f"slot{i}", [P, kmax * DIM], mybir.dt.float32)
            )
            for i in range(NBUF)
        ]
        accs = [
            stack.enter_context(
                nc.sbuf_tensor(f"acc{t}_{b}", [P, DIM], mybir.dt.float32)
            )
            for t in range(2)
            for b in range(n_blocks)
        ]
        # done sems are per (slot, queue): a sem may only be updated from one
        # SWDGE queue, while gathers rotate queues globally for 4-pair overlap.
        done = [
            [stack.enter_context(nc.semaphore(f"done{i}_{q}")) for q in range(NQ)]
            for i in range(NBUF)
        ]
        free = [stack.enter_context(nc.semaphore(f"free{i}")) for i in range(NBUF)]
        oready = stack.enter_context(nc.semaphore("oready"))
        vchain = stack.enter_context(nc.semaphore("vchain"))

        # ---- sync engine: load the packed indices
        nc.sync.dma_start(gidx_sb[:], gidx[:]).then_inc(io, 16)

        # ---- gpsimd: all gathers
        nc.gpsimd.load_library(library_config.mlp)
        nc.gpsimd.wait_ge(io, 16)
        gctr = 0          # global gather counter (queue rotation + idx cols)
        done_target = [[0] * NQ for _ in range(NBUF)]
        for j, (t, b, k) in enumerate(jobs):
            slot = j % NBUF
            if j >= NBUF:
                nc.gpsimd.wait_ge(free[slot], j // NBUF)
            kj = int(K[t, k, b])
            src = emb_cat[(t * N_CHUNKS + k) * CHUNK_ROWS:(t * N_CHUNKS + k + 1) * CHUNK_ROWS, :]
            g3 = slots[slot][:].rearrange("p (c d) -> p c d", d=DIM)
            for i in range(kj // GCOLS):
                q = gctr % NQ
                nc.gpsimd.dma_gather(
                    g3[:, i * GCOLS:(i + 1) * GCOLS, :],
                    src,
                    gidx_sb[:, gctr * (GMAX // 16):(gctr + 1) * (GMAX // 16)],
                    GMAX,
                    GMAX,
                    DIM,
                    queue_num=q,
                ).then_inc(done[slot][q], 16)
                done_target[slot][q] += 16
                gctr += 1
            # vector consumes this job when every queue sem hits its target
            jobs[j] = (t, b, k, slot, tuple(done_target[slot]), kj)

        # ---- vector: tree-reduce each job, accumulate chunks, signal outputs.
        # DVE has an 8-deep exec queue, so same-engine RAW chains need explicit
        # serialization: every op incs vchain and waits for the previous one.
        # DVE executes its own stream in order with per-op DRAIN, so RAW chains
        # on the same engine need no sems (verified on HW); the retire-marker
        # (vchain) gates cross-engine sem_incs on actual retirement.
        vc = 0
        for j, (t, b, k, slot, tgts, kj) in enumerate(jobs):
            for q in range(NQ):
                if tgts[q]:
                    nc.vector.wait_ge(done[slot][q], tgts[q])
            g = slots[slot]
            n = kj
            while n > 1:
                h = n // 2
                nc.vector.tensor_add(
                    out=g[:, : h * DIM],
                    in0=g[:, : h * DIM],
                    in1=g[:, (n - h) * DIM : n * DIM],
                )
                n -= h
            acc = accs[t * n_blocks + b]
            if k == 0:
                nc.vector.tensor_copy(out=acc[:], in_=g[:, :DIM])
            else:
                nc.vector.tensor_add(out=acc[:], in0=acc[:], in1=g[:, :DIM])
            nc.vector.tensor_copy(out=g[:, :4], in_=g[:, :4]).then_inc(vchain, 1)
            vc += 1
            nc.vector.wait_ge(vchain, vc)
            nc.vector.sem_inc(free[slot], 1)
            if k == N_CHUNKS - 1:
                nc.vector.sem_inc(oready, 1)

        # ---- sync engine: write outputs as accs complete
        m = 0
        for t in range(2):
            for b in range(n_blocks):
                m += 1
                nc.sync.wait_ge(oready, m)
                nc.sync.dma_start(
                    out=outs[t][b * P:(b + 1) * P, :],
                    in_=accs[t * n_blocks + b][:],
                ).then_inc(io, 16)
        nc.sync.wait_ge(io, 16 + m * 16)

    nc.compile()
    return nc


def _pack_core(idx_by_table, K, n_blocks):
    """Build one core's wrapped int16 index tensor.

    idx_by_table: [2, bc, SEQ] int32 core-local indices.
    Returns gidx [P, K.sum()*P//16] int16.
    """
    streams = []
    for t in range(2):
        for b in range(n_blocks):
            rows = idx_by_table[t][b * P:(b + 1) * P]  # [P, SEQ]
            for k in range(N_CHUNKS):
                kj = int(K[t, k, b])
                mask = (rows // CHUNK) == k
                local = (rows - k * CHUNK).astype(np.int64)
                order = np.argsort(~mask, axis=1, kind="stable")
                sortloc = np.take_along_axis(local, order, axis=1)
                cnt = mask.sum(axis=1)
                pad_cols = max(kj - SEQ, 0)
                if pad_cols:
                    sortloc = np.concatenate(
                        [sortloc, np.zeros((P, pad_cols), np.int64)], axis=1
                    )
                sel = sortloc[:, :kj]
                sel = np.where(np.arange(kj)[None, :] < cnt[:, None], sel, PAD_IDX)
                # stream position i -> (lane i%128, col i//128): column-major
                streams.append(sel.T.ravel())  # [kj * P]
    s = np.concatenate(streams).astype(np.int16)
    wrapped = s.reshape(-1, 16).T  # [16, n/16]
    return np.tile(wrapped, (8, 1)).copy()  # replicate across 16-partition groups


def kernel(inputs_pri, inputs_sec, emb_pri, emb_sec, _trace=False, _trace_kwargs=None):
    inputs_pri = np.ascontiguousarray(np.asarray(inputs_pri, dtype=np.int32))
    inputs_sec = np.ascontiguousarray(np.asarray(inputs_sec, dtype=np.int32))
    emb_pri = np.ascontiguousarray(np.asarray(emb_pri, dtype=np.float32))
    emb_sec = np.ascontiguousarray(np.asarray(emb_sec, dtype=np.float32))

    batch = inputs_pri.shape[0]
    bc = batch // N_CORES
    n_blocks = bc // P

    # emb_cat: [2, 4, 25001, 64] with a zero row per chunk
    emb_cat = np.zeros((2, N_CHUNKS, CHUNK_ROWS, DIM), np.float32)
    for t, emb in enumerate((emb_pri, emb_sec)):
        for k in range(N_CHUNKS):
            emb_cat[t, k, :CHUNK] = emb[k * CHUNK:(k + 1) * CHUNK]
    emb_cat = np.ascontiguousarray(emb_cat.reshape(2 * N_CHUNKS * CHUNK_ROWS, DIM))

    # per-core per-block per-chunk max token counts -> uniform K (multiple of GCOLS)
    per_core = [
        (inputs_pri[c * bc:(c + 1) * bc], inputs_sec[c * bc:(c + 1) * bc])
        for c in range(N_CORES)
    ]
    K = np.zeros((2, N_CHUNKS, n_blocks), np.int64)
    for c in range(N_CORES):
        for t in range(2):
            rows_t = per_core[c][t]
            chunk_of = rows_t // CHUNK  # [bc, SEQ]
            for b in range(n_blocks):
                blk = chunk_of[b * P:(b + 1) * P]
                for k in range(N_CHUNKS):
                    K[t, k, b] = max(K[t, k, b], (blk == k).sum(axis=1).max())
    K = ((K + GCOLS - 1) // GCOLS) * GCOLS
    K = np.maximum(K, GCOLS)

    nc = build_nc(K, n_blocks)

    in_maps = []
    for c in range(N_CORES):
        gidx = _pack_core(per_core[c], K, n_blocks)
        in_maps.append({"emb_cat": emb_cat, "gidx": gidx})

    kwargs = {}
    if _trace:
        kwargs["trace"] = True
        if _trace_kwargs:
            kwargs.update(_trace_kwargs)
    res = run_bass_kernel_spmd(nc, in_maps, list(range(N_CORES)), **kwargs)
    outs = res.results
    out_pri = np.concatenate([outs[c]["out_pri"] for c in range(N_CORES)], axis=0)
    out_sec = np.concatenate([outs[c]["out_sec"] for c in range(N_CORES)], axis=0)
    if _trace:
        return (out_pri, out_sec), res
    return out_pri, out_sec


# revision 10
# speedup vs baseline: 3.2650x; 1.2393x over previous
"""Embedding-bag (sum over sentence dim) kernel for Trainium2, 8 NeuronCores.

Problem: two embedding tables [100000, 64] f32, two index tensors [4096, 200]
int32/int64; output = (sum_s emb_pri[idx_pri[b,s]], sum_s emb_sec[idx_sec[b,s]])
as two [4096, 64] f32 tensors.

Strategy (all measured on HW):
- Data-parallel over batch: each core handles 512 rows for both tables.
- The only fast random-gather primitive is the custom-ucode `dma_gather`
  (InstDMAGatherAnt): ~2.3 ns/row with 4 parallel SWDGE queues, but indices
  are int16 and num_idxs <= 1024 per instruction. The 100k vocab is split
  into 2 chunks of 50000 rows (+1 zero pad row each) addressed with SIGNED
  int16 locals (source AP base shifted +32768 rows into the chunk); tokens
  are bucketed per chunk on the host and padded per 128-row block to the
  block's max per-row chunk count (pads point at the zero row, contributing
  0). The ucode drops trailing negative indices per stream, so the packer
  permutes lane 127's columns to keep every gather's last slot >= 0.
- dma_gather places stream position i at SBUF [i%128, i//128, :], so a
  stream packed column-major (lane p = batch row p of the block) lands each
  batch row's tokens in one partition; an in-place DVE binary-tree add then
  reduces the K token-columns to [128, 64], accumulated over the 4 chunks.
- Raw bacc program with an explicit semaphore pipeline:
    sync:   gidx load, output DMAs
    gpsimd: 4-queue rotated dma_gathers into a ring of dest tiles
    vector: tree-reduce + chunk accumulation, slot recycling
"""

import sys

if "/opt/trn_rl_repo" not in sys.path:
    sys.path.insert(0, "/opt/trn_rl_repo")

from contextlib import ExitStack

import numpy as np

import concourse.bacc as bacc
import concourse.bass as bass
import concourse.mybir as mybir
from concourse import library_config
from concourse.bass_utils import run_bass_kernel_spmd

N_CORES = 8
P = 128
VOCAB = 100000
SEQ = 200
DIM = 64
BATCH = 4096

N_CHUNKS = 2
CHUNK = VOCAB // N_CHUNKS          # 50000 (signed int16 indexing, base +32768)
CHUNK_ROWS = CHUNK + 1             # + zero pad row
BASE_SHIFT = 32768                 # in_ap base is shifted this many rows in
PAD_IDX = CHUNK - BASE_SHIFT       # local index of the zero row (positive)
GMAX = 1024                        # max num_idxs per dma_gather (HW ring limit)
GCOLS = GMAX // P                  # 8 dest columns per gather
NBUF = 4                           # dest tile ring depth
NQ = 4                             # SWDGE queues


def build_nc(K, n_blocks):
    """Build the SPMD per-core program.

    K: int array [2, N_CHUNKS, n_blocks] -- token columns per job, each a
       multiple of GCOLS; identical across cores (max over cores).
    """
    kmax = int(K.max())
    total_cols = int(K.sum())
    idx_cols = total_cols * P // 16  # int16 columns of the wrapped index tensor

    nc = bacc.Bacc("TRN2", debug=False, num_swdge_queues=NQ)

    emb_cat = nc.dram_tensor(
        "emb_cat", [2 * N_CHUNKS * CHUNK_ROWS, DIM], mybir.dt.float32,
        kind="ExternalInput",
    )
    gidx = nc.dram_tensor("gidx", [P, idx_cols], mybir.dt.int16, kind="ExternalInput")
    out_pri = nc.dram_tensor("out_pri", [n_blocks * P, DIM], mybir.dt.float32, kind="ExternalOutput")
    out_sec = nc.dram_tensor("out_sec", [n_blocks * P, DIM], mybir.dt.float32, kind="ExternalOutput")
    outs = (out_pri, out_sec)

    # job order: (T, b) outer, chunk k inner so chunk partials accumulate
    jobs = [(t, b, k) for t in range(2) for b in range(n_blocks) for k in range(N_CHUNKS)]

    with (
        nc.Block() as _block,
        nc.sbuf_tensor("gidx_sb", [P, idx_cols], mybir.dt.int16) as gidx_sb,
        nc.semaphore("io") as io,
        ExitStack() as stack,
    ):
        slots = [
            stack.enter_context(
                nc.sbuf_tensor(f"slot{i}", [P, kmax * DIM], mybir.dt.float32)
            )
            for i in range(NBUF)
        ]
        accs = [
            stack.enter_context(
                nc.sbuf_tensor(f"acc{t}_{b}", [P, DIM], mybir.dt.float32)
            )
            for t in range(2)
            for b in range(n_blocks)
        ]
        # done sems are per (slot, queue): a sem may only be updated from one
        # SWDGE queue, while gathers rotate queues globally for 4-pair overlap.
        done = [
            [stack.enter_context(nc.semaphore(f"done{i}_{q}")) for q in range(NQ)]
            for i in range(NBUF)
        ]
        free = [stack.enter_context(nc.semaphore(f"free{i}")) for i in range(NBUF)]
        oready = stack.enter_context(nc.semaphore("oready"))
        vchain = stack.enter_context(nc.semaphore("vchain"))

        # ---- sync engine: load the packed indices
        nc.sync.dma_start(gidx_sb[:], gidx[:]).then_inc(io, 16)

        # ---- gpsimd: all gathers
        nc.gpsimd.load_library(library_config.mlp)
        nc.gpsimd.wait_ge(io, 16)
        gctr = 0          # global gather counter (queue rotation + idx cols)
        done_target = [[0] * NQ for _ in range(NBUF)]
        for j, (t, b, k) in enumerate(jobs):
            slot = j % NBUF
            if j >= NBUF:
                nc.gpsimd.wait_ge(free[slot], j // NBUF)
            kj = int(K[t, k, b])
            base = (t * N_CHUNKS + k) * CHUNK_ROWS + BASE_SHIFT
            src = emb_cat[base:(t * N_CHUNKS + k + 1) * CHUNK_ROWS, :]
            g3 = slots[slot][:].rearrange("p (c d) -> p c d", d=DIM)
            for i in range(kj // GCOLS):
                q = gctr % NQ
                nc.gpsimd.dma_gather(
                    g3[:, i * GCOLS:(i + 1) * GCOLS, :],
                    src,
                    gidx_sb[:, gctr * (GMAX // 16):(gctr + 1) * (GMAX // 16)],
                    GMAX,
                    GMAX,
                    DIM,
                    queue_num=q,
                ).then_inc(done[slot][q], 16)
                done_target[slot][q] += 16
                gctr += 1
            # vector consumes this job when every queue sem hits its target
            jobs[j] = (t, b, k, slot, tuple(done_target[slot]), kj)

        # ---- vector: tree-reduce each job, accumulate chunks, signal outputs.
        # DVE has an 8-deep exec queue, so same-engine RAW chains need explicit
        # serialization: every op incs vchain and waits for the previous one.
        # DVE executes its own stream in order with per-op DRAIN, so RAW chains
        # on the same engine need no sems (verified on HW); the retire-marker
        # (vchain) gates cross-engine sem_incs on actual retirement.
        vc = 0
        for j, (t, b, k, slot, tgts, kj) in enumerate(jobs):
            for q in range(NQ):
                if tgts[q]:
                    nc.vector.wait_ge(done[slot][q], tgts[q])
            g = slots[slot]
            n = kj
            while n > 1:
                h = n // 2
                nc.vector.tensor_add(
                    out=g[:, : h * DIM],
                    in0=g[:, : h * DIM],
                    in1=g[:, (n - h) * DIM : n * DIM],
                )
                n -= h
            acc = accs[t * n_blocks + b]
            if k == 0:
                nc.vector.tensor_copy(out=acc[:], in_=g[:, :DIM])
            else:
                nc.vector.tensor_add(out=acc[:], in0=acc[:], in1=g[:, :DIM])
            nc.vector.tensor_copy(out=g[:, :4], in_=g[:, :4]).then_inc(vchain, 1)
            vc += 1
            nc.vector.wait_ge(vchain, vc)
            nc.vector.sem_inc(free[slot], 1)
            if k == N_CHUNKS - 1:
                nc.vector.sem_inc(oready, 1)

        # ---- sync engine: write outputs as accs complete
        m = 0
        for t in range(2):
            for b in range(n_blocks):
                m += 1
                nc.sync.wait_ge(oready, m)
                nc.sync.dma_start(
                    out=outs[t][b * P:(b + 1) * P, :],
                    in_=accs[t * n_blocks + b][:],
                ).then_inc(io, 16)
        nc.sync.wait_ge(io, 16 + m * 16)

    nc.compile()
    return nc


KK_GCOLS = GCOLS


def _pack_core(idx_by_table, K, n_blocks):
    """Build one core's wrapped int16 index tensor.

    idx_by_table: [2, bc, SEQ] int32 core-local indices.
    Returns gidx [P, K.sum()*P//16] int16.
    """
    streams = []
    for t in range(2):
        for b in range(n_blocks):
            rows = idx_by_table[t][b * P:(b + 1) * P]  # [P, SEQ]
            for k in range(N_CHUNKS):
                kj = int(K[t, k, b])
                mask = (rows // CHUNK) == k
                local = (rows - k * CHUNK - BASE_SHIFT).astype(np.int64)
                order = np.argsort(~mask, axis=1, kind="stable")
                sortloc = np.take_along_axis(local, order, axis=1)
                cnt = mask.sum(axis=1)
                pad_cols = max(kj - SEQ, 0)
                if pad_cols:
                    sortloc = np.concatenate(
                        [sortloc, np.zeros((P, pad_cols), np.int64)], axis=1
                    )
                sel = sortloc[:, :kj]
                sel = np.where(np.arange(kj)[None, :] < cnt[:, None], sel, PAD_IDX)
                # ucode drops TRAILING negative indices per 1024-stream: the
                # last slot of each gather (lane 127, col 8g+7) must be >= 0.
                # Column order within a lane doesn't affect the sum: permute.
                row127 = sel[127].copy()
                for g in range(kj // KK_GCOLS):
                    last = g * KK_GCOLS + KK_GCOLS - 1
                    if row127[last] < 0:
                        cand = [j for j in range(kj)
                                if row127[j] >= 0 and j % KK_GCOLS != KK_GCOLS - 1]
                        assert cand, "no non-negative index available for lane 127"
                        j = cand[0]
                        row127[last], row127[j] = row127[j], row127[last]
                sel[127] = row127
                # stream position i -> (lane i%128, col i//128): column-major
                streams.append(sel.T.ravel())  # [kj * P]
    s = np.concatenate(streams).astype(np.int16)
    wrapped = s.reshape(-1, 16).T  # [16, n/16]
    return np.tile(wrapped, (8, 1)).copy()  # replicate across 16-partition groups


def kernel(inputs_pri, inputs_sec, emb_pri, emb_sec, _trace=False, _trace_kwargs=None):
    inputs_pri = np.ascontiguousarray(np.asarray(inputs_pri, dtype=np.int32))
    inputs_sec = np.ascontiguousarray(np.asarray(inputs_sec, dtype=np.int32))
    emb_pri = np.ascontiguousarray(np.asarray(emb_pri, dtype=np.float32))
    emb_sec = np.ascontiguousarray(np.asarray(emb_sec, dtype=np.float32))

    batch = inputs_pri.shape[0]
    bc = batch // N_CORES
    n_blocks = bc // P

    # emb_cat: [2, 4, 25001, 64] with a zero row per chunk
    emb_cat = np.zeros((2, N_CHUNKS, CHUNK_ROWS, DIM), np.float32)
    for t, emb in enumerate((emb_pri, emb_sec)):
        for k in range(N_CHUNKS):
            emb_cat[t, k, :CHUNK] = emb[k * CHUNK:(k + 1) * CHUNK]
    emb_cat = np.ascontiguousarray(emb_cat.reshape(2 * N_CHUNKS * CHUNK_ROWS, DIM))

    # per-core per-block per-chunk max token counts -> uniform K (multiple of GCOLS)
    per_core = [
        (inputs_pri[c * bc:(c + 1) * bc], inputs_sec[c * bc:(c + 1) * bc])
        for c in range(N_CORES)
    ]
    K = np.zeros((2, N_CHUNKS, n_blocks), np.int64)
    for c in range(N_CORES):
        for t in range(2):
            rows_t = per_core[c][t]
            chunk_of = rows_t // CHUNK  # [bc, SEQ]
            for b in range(n_blocks):
                blk = chunk_of[b * P:(b + 1) * P]
                for k in range(N_CHUNKS):
                    K[t, k, b] = max(K[t, k, b], (blk == k).sum(axis=1).max())
    K = ((K + GCOLS - 1) // GCOLS) * GCOLS
    K = np.maximum(K, GCOLS)

    nc = build_nc(K, n_blocks)

    in_maps = []
    for c in range(N_CORES):
        gidx = _pack_core(per_core[c], K, n_blocks)
        in_maps.append({"emb_cat": emb_cat, "gidx": gidx})

    kwargs = {}
    if _trace:
        kwargs["trace"] = True
        if _trace_kwargs:
            kwargs.update(_trace_kwargs)
    res = run_bass_kernel_spmd(nc, in_maps, list(range(N_CORES)), **kwargs)
    outs = res.results
    out_pri = np.concatenate([outs[c]["out_pri"] for c in range(N_CORES)], axis=0)
    out_sec = np.concatenate([outs[c]["out_sec"] for c in range(N_CORES)], axis=0)
    if _trace:
        return (out_pri, out_sec), res
    return out_pri, out_sec
